# revision 17
# baseline (speedup 1.0000x reference)
"""Pointer2D banded span-softmax kernel for 8 Trainium2 NeuronCores.

Math: reference computes, per batch b,
    softmax_n( dot(start[b, si[n]], w) + dot(end[b, ei[n]], w) + bias
               - 1e7 * (1 - mask[si[n]]*mask[ei[n]]) )
over the band si[n]=i, ei[n]=i+k (0<=k<8, i+k<512), N=4068.

Since Dense(1) is linear, we compute per-position scores
    s[i] = start[b,i,:] . w      e[j] = end[b,j,:] . w
in one streaming pass over the embeddings (the memory roofline), then
build the dense 512x8 banded logit grid from the tiny score vectors,
mask, exp, normalize.  Host slices the 4096-dense grid down to the 4068
valid band entries.

Sharding: pure data parallel, 4 batches per core.

Device layout trick: X tiles are loaded position-interleaved
(rows t, t+4, t+8, ...) so the matvec's accum column t on partition p
holds position 4p+t.  The banded grid
    L[p, 8c+k] = s[4p+c] + e[4p+c+k]
is then expressible with per-partition tensor_scalar ops on an 11-wide
sliding-window tile, built from the 4-wide score tile with two tiny
partition-shift SBUF->SBUF DMAs.

Masking: pen = (ms*me)*1e7 - 1e7 is computed BEFORE touching the
logits (1e7-1e7 cancels exactly in f32 for unmasked entries; adding
1e7 to a logit would quantize it to ULP(1e7)=1.0).  Logits are clamped
at -200 so exp underflows masked/out-of-band entries to exactly 0.
"""

import sys

for _p in ("/opt/trn_rl_repo",):
    if _p not in sys.path:
        sys.path.insert(0, _p)

import numpy as np

import concourse.bass as bass
import concourse.mybir as mybir
from concourse.bacc import Bacc
from concourse.tile import TileContext
from concourse.bass_utils import run_bass_kernel_spmd

S = 512          # sequence length
A = 8            # max answer length (band width)
B = 32           # batch
D = 1024         # hidden
NCORES = 8
BLOC = B // NCORES   # batches per core = 4
GRID = S * A         # 4096 dense band grid per batch
DT = mybir.dt.float32

_ALU = mybir.AluOpType
_ACT = mybir.ActivationFunctionType


def _one_batch(nc, bi, emb, maskf, out, p_x, p_junk, p_small, p_psum,
               ones_col, ones_row, w_b, E_all, red, O_all, bias_val):
    # ---- streaming matvec: s,e scores ----
    s_buf = p_small.tile([128, 4], DT, name=f"s_buf_{bi}", tag="s_buf")
    e4 = p_small.tile([128, 4], DT, name=f"e4_{bi}", tag="e4")
    for t in range(4):
        xt = p_x.tile([128, D], DT, name=f"x_{bi}_{t}", tag="xt")
        # positions t, t+4, ..., t+508  (partition p = position 4p+t)
        src = emb[bi].rearrange("(p four) d -> four p d", four=4)[t]
        nc.sync.dma_start(out=xt[:, :], in_=src)
        # DVE: elementwise X*w for both halves
        prod_s = p_junk.tile([128, S], DT, name="prod_s", tag="prod_s")
        nc.vector.tensor_mul(prod_s[:, :], xt[:, 0:S], w_b[:, :])
        prod_e = p_junk.tile([128, S], DT, name="prod_e", tag="prod_e")
        nc.vector.tensor_mul(prod_e[:, :], xt[:, S:D], w_b[:, :])
        # reduce s-half on ACT (Copy + accum), e-half on DVE, to balance
        # engine load
        junk_a = p_junk.tile([128, S], DT, name="junk_a", tag="junk_a")
        nc.scalar.activation(
            out=junk_a[:, :],
            in_=prod_s[:, :],
            func=_ACT.Copy,
            accum_out=s_buf[:, t : t + 1],
        )
        nc.vector.tensor_reduce(
            out=e4[:, t : t + 1],
            in_=prod_e[:, :],
            axis=mybir.AxisListType.X,
            op=_ALU.add,
        )

    # ---- 11-wide sliding windows: e_buf[p,j] = e[4p+j] ----
    e_buf = p_small.tile([128, 11], DT, name=f"e_buf_{bi}", tag="e_buf")
    nc.vector.memset(e_buf[:, 4:11], 0.0)
    nc.vector.tensor_copy(out=e_buf[:, 0:4], in_=e4[:, :])
    nc.sync.dma_start(out=e_buf[0:127, 4:8], in_=e4[1:128, 0:4])
    nc.sync.dma_start(out=e_buf[0:126, 8:11], in_=e4[2:128, 0:3])

    ms = p_small.tile([128, 4], DT, name=f"ms_{bi}", tag="ms")
    nc.sync.dma_start(
        out=ms[:, :], in_=maskf[bi].rearrange("(p c) -> p c", c=4)
    )
    me = p_small.tile([128, 11], DT, name=f"me_{bi}", tag="me")
    nc.vector.memset(me[:, 4:11], 0.0)
    nc.vector.tensor_copy(out=me[:, 0:4], in_=ms[:, :])
    nc.sync.dma_start(out=me[0:127, 4:8], in_=ms[1:128, 0:4])
    nc.sync.dma_start(out=me[0:126, 8:11], in_=ms[2:128, 0:3])

    # ---- banded logit grid [128, 32]: n = 32p + 8c + k ----
    L = p_small.tile([128, 32], DT, name=f"L_{bi}", tag="L")
    PM = p_small.tile([128, 32], DT, name=f"PM_{bi}", tag="PM")
    for c in range(4):
        nc.vector.tensor_scalar(
            out=L[:, 8 * c : 8 * c + 8],
            in0=e_buf[:, c : c + 8],
            scalar1=s_buf[:, c : c + 1],
            scalar2=None,
            op0=_ALU.add,
        )
        # PM = mask_s * mask_e * 1e7
        nc.vector.tensor_scalar(
            out=PM[:, 8 * c : 8 * c + 8],
            in0=me[:, c : c + 8],
            scalar1=ms[:, c : c + 1],
            scalar2=1.0e7,
            op0=_ALU.mult,
            op1=_ALU.mult,
        )

    # PEN = PM - 1e7: exactly 0.0 for unmasked entries
    PEN = p_small.tile([128, 32], DT, name=f"PEN_{bi}", tag="PEN")
    nc.vector.tensor_scalar(
        out=PEN[:, :], in0=PM[:, :], scalar1=-1.0e7, scalar2=None, op0=_ALU.add
    )
    L2 = p_small.tile([128, 32], DT, name=f"L2_{bi}", tag="L2")
    nc.vector.tensor_tensor(out=L2[:, :], in0=L[:, :], in1=PEN[:, :], op=_ALU.add)
    # L3 = max(L2 + bias, -200); exp(-200) underflows to exact 0
    L3 = p_small.tile([128, 32], DT, name=f"L3_{bi}", tag="L3")
    nc.vector.tensor_scalar(
        out=L3[:, :],
        in0=L2[:, :],
        scalar1=bias_val,
        scalar2=-200.0,
        op0=_ALU.add,
        op1=_ALU.max,
    )

    # exp with fused per-partition row-sum
    nc.scalar.activation(
        out=E_all[:, 32 * bi : 32 * bi + 32],
        in_=L3[:, :],
        func=_ACT.Exp,
        bias=0.0,
        scale=1.0,
        accum_out=red[:, bi : bi + 1],
    )

    # ---- denominator: sum over partitions via PE ----
    # (every matmul operand goes through a DVE op first: PE instructions
    # here can carry only ONE sync wait, so deps must collapse onto DVE)
    red_c = p_small.tile([128, 1], DT, name=f"red_c_{bi}", tag="red_c")
    nc.vector.tensor_copy(out=red_c[:, :], in_=red[:, bi : bi + 1])
    den_ps = p_psum.tile([1, 1], DT, name=f"den_ps_{bi}", tag="den_ps")
    nc.tensor.matmul(
        den_ps[:, :], lhsT=ones_col[:, :], rhs=red_c[:, :],
        start=True, stop=True,
    )
    den_sb = p_small.tile([1, 1], DT, name=f"den_sb_{bi}", tag="den_sb")
    nc.vector.tensor_copy(out=den_sb[:, :], in_=den_ps[:, :])
    bc_ps = p_psum.tile([128, 1], DT, name=f"bc_ps_{bi}", tag="bc_ps")
    nc.tensor.matmul(
        bc_ps[:, :], lhsT=ones_row[:, :], rhs=den_sb[:, :],
        start=True, stop=True,
    )
    inv = p_small.tile([128, 1], DT, name=f"inv_{bi}", tag="inv")
    nc.vector.reciprocal(inv[:, :], bc_ps[:, :])

    nc.vector.tensor_scalar(
        out=O_all[:, 32 * bi : 32 * bi + 32],
        in0=E_all[:, 32 * bi : 32 * bi + 32],
        scalar1=inv[:, 0:1],
        scalar2=None,
        op0=_ALU.mult,
    )
    nc.sync.dma_start(
        out=out[bi].rearrange("(p f) -> p f", f=32),
        in_=O_all[:, 32 * bi : 32 * bi + 32],
    )


def _build_nc(bias_val: float, reps: int = 1) -> bass.Bass:
    """reps>1 wraps the per-batch pipeline in a tc.For_i hardware loop —
    used only for wall-clock slope timing in the test harness."""
    nc = Bacc()

    emb = nc.declare_dram_parameter("emb", [BLOC, S, D], DT, isOutput=False)
    maskf = nc.declare_dram_parameter("maskf", [BLOC, S], DT, isOutput=False)
    wv = nc.declare_dram_parameter("wv", [1, S], DT, isOutput=False)
    out = nc.declare_dram_parameter("out", [BLOC, GRID], DT, isOutput=True)

    with TileContext(nc) as tc:
        with (
            tc.tile_pool(name="p_x", bufs=8) as p_x,
            tc.tile_pool(name="p_junk", bufs=2) as p_junk,
            tc.tile_pool(name="p_small", bufs=2) as p_small,
            tc.tile_pool(name="p_pers", bufs=1) as p_pers,
            tc.tile_pool(name="p_psum1", bufs=1, space="PSUM") as p_psum1,
            tc.tile_pool(name="p_psum", bufs=2, space="PSUM") as p_psum,
        ):
            # ---- one-time setup ----
            ones_col = p_pers.tile([128, 1], DT)
            nc.vector.memset(ones_col[:, :], 1.0)
            ones_row = p_pers.tile([1, 128], DT)
            nc.vector.memset(ones_row[:, :], 1.0)

            # broadcast w to all 128 partitions via PE: ones[1,128].T @ w[1,512]
            w_row = p_pers.tile([1, S], DT)
            nc.sync.dma_start(out=w_row[:, :], in_=wv[:, :])
            w_rowc = p_pers.tile([1, S], DT)
            nc.vector.tensor_copy(out=w_rowc[:, :], in_=w_row[:, :])
            psum_w = p_psum1.tile([128, S], DT, name="psum_w", tag="psum_w")
            nc.tensor.matmul(
                psum_w[:, :], lhsT=ones_row[:, :], rhs=w_rowc[:, :],
                start=True, stop=True,
            )
            w_b = p_pers.tile([128, S], DT)
            nc.scalar.copy(out=w_b[:, :], in_=psum_w[:, :])

            E_all = p_pers.tile([128, 32 * BLOC], DT)
            red = p_pers.tile([128, BLOC], DT)
            O_all = p_pers.tile([128, 32 * BLOC], DT)

            def _batch_body():
                for bi in range(BLOC):
                    _one_batch(nc, bi, emb, maskf, out, p_x, p_junk, p_small,
                               p_psum, ones_col, ones_row, w_b, E_all, red,
                               O_all, bias_val)

            if reps == 1:
                _batch_body()
            else:
                with tc.For_i(0, reps, 1):
                    _batch_body()

    nc.finalize()
    return nc


_VALID = None


def _valid_mask() -> np.ndarray:
    global _VALID
    if _VALID is None:
        n = np.arange(GRID)
        _VALID = (n >> 3) + (n & 7) < S
    return _VALID


def _in_maps(embeddings, mask, w):
    in_maps = []
    for c in range(NCORES):
        sl = slice(BLOC * c, BLOC * (c + 1))
        in_maps.append(
            {
                "emb": np.ascontiguousarray(embeddings[sl]),
                "maskf": np.ascontiguousarray(mask[sl]),
                "wv": w,
            }
        )
    return in_maps


def kernel(embeddings, token_type_ids, attention_mask, W, b) -> np.ndarray:
    embeddings = np.asarray(embeddings, dtype=np.float32)
    mask = (
        np.asarray(token_type_ids, dtype=np.int32)
        * np.asarray(attention_mask, dtype=np.int32)
    ).astype(np.float32)
    w = np.ascontiguousarray(np.asarray(W, dtype=np.float32).reshape(-1))[None, :]
    bias_val = float(np.asarray(b, dtype=np.float32).reshape(-1)[0])

    nc = _build_nc(bias_val)
    res = run_bass_kernel_spmd(nc, _in_maps(embeddings, mask, w), list(range(NCORES)))
    dense = np.concatenate([res.results[c]["out"] for c in range(NCORES)], axis=0)
    return np.ascontiguousarray(dense[:, _valid_mask()]).astype(np.float32)


if __name__ == "__main__":
    rng = np.random.default_rng(0)
    inputs = {
        "embeddings": rng.standard_normal((B, S, D), dtype=np.float32),
        "token_type_ids": rng.integers(0, 2, (B, S)).astype(np.int32),
        "attention_mask": rng.integers(0, 2, (B, S)).astype(np.int32),
        "W": (rng.standard_normal((S, 1)) * 0.02).astype(np.float32),
        "b": np.zeros((1,), np.float32),
    }
    out = kernel(**inputs)
    print(out.shape, out.dtype, out.sum(axis=-1)[:4])
